# revision 12
# baseline (speedup 1.0000x reference)
"""AutoCorrelation (Autoformer) Trainium2 kernel, 8-core data-parallel over batch.

Per core (one batch b), computes mean_value[b, tau] = (1/(H*E)) sum_c
circ-crosscorr(k[:,c], q[:,c])[tau] via a 16-subsequence DFT-128 decomposition
(t = 16u + r), then AllReduce over cores -> top-7 delays -> softmax weights ->
7-tap shifted weighted sum of V.

v2 layout/engine plan (all heavy matmuls fp16/bf16, 1 cycle/row on PE):
  - host prep: q/k pre-gathered per 128-channel chunk as [u, r, c] fp16 (K with
    r reversed, which the diagonal-skew sums require), V pre-transposed [c, t]
    bf16, output left as [c, t] fp32 and un-transposed on host.
  - stage A (spectra): per (ch, r) the data block [u=128, c=128] is the
    stationary operand (128 contiguous fp16 cols -> FWL), streaming a packed
    DFT matrix: Q side 128 cols (re|im), K side 192 cols (im|re|-im). PSUM ->
    fp16 spectra in SBUF laid out [c, (fp, r)] so stage-P slices are contiguous.
  - stage P (cross spectra): per (f, ch) TWO accumulating matmuls produce
    P_re and P_im directly in PSUM:
      [Qre]^T [Kre | -Kim] + [Qim]^T [Kim | Kre] = [P_re | P_im]
    (the -Kim slab comes for free from stage A's K DFT matrix).
  - diagonal sums: DRAM skew write / aligned readback (as before), but the
    16-partition fold is a DVE segmented tensor_reduce over the readback laid
    out [j, (f, a)] - no gpsimd involvement.
  - IDFT-128 (fp32, exact) -> mean_value; AllReduce; on-device top-7 +
    softmax; 7-tap PSUM-accumulated taps on PE with scaled-identity
    stationaries over doubled V^T; output DMA'd straight from PSUM to DRAM.
"""

import os
import sys
import numpy as np

for p in ("/opt/trn_rl_repo",):
    if p not in sys.path and os.path.isdir(p):
        sys.path.insert(0, p)

import ml_dtypes
import concourse.bass as bass
import concourse.bacc as bacc
import concourse.tile as tile
import concourse.mybir as mybir
from concourse import bass_utils

F32 = mybir.dt.float32
F16 = mybir.dt.float16
BF16 = mybir.dt.bfloat16
U32 = mybir.dt.uint32
AL = mybir.AluOpType

B, L, H, E = 8, 2048, 8, 64
C = H * E            # 512
U, R = 128, 16       # L = U * R ; t = 16*u + r
NF = 65              # rfft freqs of DFT-128 (0..64)
NCH = C // 128       # 4 channel chunks
NCORES = 8
TOPK = 7
SKROW = 3121         # skew row pitch (DRAM floats per a-row)


def _consts():
    u = np.arange(U)
    f65 = np.arange(NF)
    f2 = np.arange(1, 64)

    # Q-side packed DFT: cols 0..64 = cos, 65..127 = -sin (f=1..63)
    wdftq = np.zeros((U, 128), dtype=np.float16)
    wdftq[:, :NF] = np.cos(2 * np.pi * np.outer(u, f65) / U)
    wdftq[:, NF:] = -np.sin(2 * np.pi * np.outer(u, f2) / U)

    # K-side packed DFT: cols 0..62 = -sin (im, f=1..63), 63..127 = cos
    # (re, f=0..64), 128..190 = +sin (-im, f=1..63), col 191 zero pad.
    wdftk = np.zeros((U, 192), dtype=np.float16)
    wdftk[:, 0:63] = -np.sin(2 * np.pi * np.outer(u, f2) / U)
    wdftk[:, 63:128] = np.cos(2 * np.pi * np.outer(u, f65) / U)
    wdftk[:, 128:191] = np.sin(2 * np.pi * np.outer(u, f2) / U)

    # IDFT on packed spectrum -> mean_value (incl 2x Hermitian weight, 1/(U*C))
    widft = np.zeros((128, U), dtype=np.float32)
    v = np.arange(U)
    scale = np.ones(NF)
    scale[1:64] = 2.0
    norm = 1.0 / (U * C)
    widft[:NF, :] = (scale[:, None] * np.cos(2 * np.pi * np.outer(f65, v) / U)) * norm
    widft[NF:, :] = (-2.0 * np.sin(2 * np.pi * np.outer(f2, v) / U)) * norm

    twv1 = np.zeros((128, 1), dtype=np.float32)
    twv2 = np.zeros((128, 1), dtype=np.float32)
    twv1[:NF, 0] = np.cos(2 * np.pi * f65 / U)
    twv2[:NF, 0] = -np.sin(2 * np.pi * f65 / U)
    twv1[NF:, 0] = np.cos(2 * np.pi * f2 / U)
    twv2[NF:, 0] = np.sin(2 * np.pi * f2 / U)
    widft_l1 = (twv1 * widft).astype(np.float32)
    widft_l2 = (twv2 * widft).astype(np.float32)

    identb = np.eye(128, dtype=ml_dtypes.bfloat16)
    skz = np.zeros((16 * SKROW,), dtype=np.float32)
    return wdftq, wdftk, widft, widft_l1, widft_l2, identb, skz


def build_kernel(nc, no_collective=False):
    qx_ext = nc.dram_tensor("qx", [NCH * 128, R * 128], F16, kind="ExternalInput")
    kx_ext = nc.dram_tensor("kx", [NCH * 128, R * 128], F16, kind="ExternalInput")
    vt_ext = nc.dram_tensor("vt", [C, L], BF16, kind="ExternalInput")
    wdftq_ext = nc.dram_tensor("wdftq", [U, 128], F16, kind="ExternalInput")
    wdftk_ext = nc.dram_tensor("wdftk", [U, 192], F16, kind="ExternalInput")
    widft_ext = nc.dram_tensor("widft", [128, U], F32, kind="ExternalInput")
    widftl1_ext = nc.dram_tensor("widftl1", [128, U], F32, kind="ExternalInput")
    widftl2_ext = nc.dram_tensor("widftl2", [128, U], F32, kind="ExternalInput")
    identb_ext = nc.dram_tensor("identb", [128, 128], BF16, kind="ExternalInput")
    skz_ext = nc.dram_tensor("skz", [16 * SKROW], F32, kind="ExternalInput")
    out_ext = nc.dram_tensor("out", [C, L], F32, kind="ExternalOutput")

    with tile.TileContext(nc) as tc:
        with (
            tc.tile_pool(name="const", bufs=1) as constp,
            tc.tile_pool(name="spec", bufs=1) as specp,
            tc.tile_pool(name="stage", bufs=2) as stagep,
            tc.tile_pool(name="vt", bufs=1) as vtp,
            tc.tile_pool(name="small", bufs=1) as smallp,
            tc.tile_pool(name="dg", bufs=2) as dgp,
            tc.tile_pool(name="psA", bufs=4, space="PSUM") as psA,
            tc.tile_pool(name="psP", bufs=2, space="PSUM") as psP,
            tc.tile_pool(name="psO", bufs=2, space="PSUM") as psO,
            tc.tile_pool(name="dram", bufs=1, space="DRAM") as dramp,
        ):
            # ---- constants ----
            wdftq_sb = constp.tile([U, 128], F16, tag="wdftq")
            wdftk_sb = constp.tile([U, 192], F16, tag="wdftk")
            widft_sb = constp.tile([128, U], F32, tag="widft")
            widftl1_sb = constp.tile([128, U], F32, tag="widftl1")
            widftl2_sb = constp.tile([128, U], F32, tag="widftl2")
            identb_sb = constp.tile([128, 128], BF16, tag="identb")
            nc.sync.dma_start(wdftq_sb[:], wdftq_ext.ap())
            nc.sync.dma_start(wdftk_sb[:], wdftk_ext.ap())
            nc.sync.dma_start(widft_sb[:], widft_ext.ap())
            nc.sync.dma_start(widftl1_sb[:], widftl1_ext.ap())
            nc.sync.dma_start(widftl2_sb[:], widftl2_ext.ap())
            nc.sync.dma_start(identb_sb[:], identb_ext.ap())

            # ---- stage A: subsequence spectra, fp16, layout [c, (fp, r)] ----
            QS = [specp.tile([128, 128 * R], F16, tag=f"qs{ch}", name=f"qs{ch}")
                  for ch in range(NCH)]
            KS = [specp.tile([128, 192 * R], F16, tag=f"ks{ch}", name=f"ks{ch}")
                  for ch in range(NCH)]
            qsrc = qx_ext.ap().rearrange("(g p) w -> g p w", p=128)
            ksrc = kx_ext.ap().rearrange("(g p) w -> g p w", p=128)
            def _copy_alt(i, dst, src):
                if i % 2 == 0:
                    nc.vector.tensor_copy(dst, src)
                else:
                    nc.scalar.activation(
                        dst, src, mybir.ActivationFunctionType.Copy)
            cp_i = 0
            for ch in range(NCH):
                xq = stagep.tile([128, R * 128], F16, tag="xq", name="xq")
                nc.sync.dma_start(xq[:], qsrc[ch])
                for r4 in range(R // 4):
                    ps = psA.tile([128, 512], F32, tag="psa")
                    for j in range(4):
                        r = r4 * 4 + j
                        nc.tensor.matmul(
                            ps[:, j * 128:(j + 1) * 128],
                            xq[:, r * 128:(r + 1) * 128],
                            wdftq_sb[:], start=True, stop=True)
                    _copy_alt(cp_i, QS[ch][:, r4 * 512:(r4 + 1) * 512], ps[:])
                    cp_i += 1
                xk = stagep.tile([128, R * 128], F16, tag="xk", name="xk")
                nc.sync.dma_start(xk[:], ksrc[ch])
                for r2 in range(R // 2):
                    psk_t = psA.tile([128, 512], F32, tag="psa")
                    psk = psk_t[:, 0:384]
                    for j in range(2):
                        r = r2 * 2 + j
                        nc.tensor.matmul(
                            psk[:, j * 192:(j + 1) * 192],
                            xk[:, r * 128:(r + 1) * 128],
                            wdftk_sb[:], start=True, stop=True)
                    _copy_alt(cp_i, KS[ch][:, r2 * 384:(r2 + 1) * 384], psk[:])
                    cp_i += 1

            # views [c, h, y, r]: Q h in {re, im}, K h in {im, re, -im}
            QSv = [QS[ch].rearrange("c (r h y) -> c h y r", h=2, y=64)
                   for ch in range(NCH)]
            KSv = [KS[ch].rearrange("c (r h y) -> c h y r", h=3, y=64)
                   for ch in range(NCH)]

            # skew scratch in DRAM, pre-zeroed from a host zeros buffer
            skd_re = dramp.tile([16 * SKROW], F32, tag="skdre", name="skd_re")
            skd_im = dramp.tile([16 * SKROW], F32, tag="skdim", name="skd_im")
            nc.sync.dma_start(skd_re[:], skz_ext.ap())
            nc.sync.dma_start(skd_im[:], skz_ext.ap())

            # ---- V^T load (scalar queue; streams under stage P etc) ----
            VT = [vtp.tile([128, 2 * L], BF16, tag=f"vt{ch}", name=f"vt{ch}")
                  for ch in range(NCH)]
            vsrc = vt_ext.ap().rearrange("(g p) t -> g p t", p=128)
            for ch in range(NCH):
                nc.scalar.dma_start(VT[ch][:, 0:L], vsrc[ch])
                nc.scalar.dma_start(VT[ch][:, L:2 * L], vsrc[ch])

            # ---- stage P + diagonal sums ----
            # PP[a, f*32 + (0:16 re | 16:32 im)] with a = Q's r, b = K's slot
            PP = specp.tile([16, NF * 32], F32, tag="pp", name="pp")
            PPv = PP.rearrange("a (f h b) -> a f h b", h=2, b=16)
            # readback target: partition = packed f (re 0..64 | im f=1..63 at
            # 64+f), free = (a, j); folding a is then 4 free-dim slice adds.
            DG4 = dgp.tile([128, 16 * 32], F32, tag="dg4", name="dg4", bufs=1)

            def q_slice(ch, f, half):
                # half 0 = re, 1 = im (packed -sin sums)
                if f == 64:
                    assert half == 0
                    return QSv[ch][:, 1:2, 0, :]
                return QSv[ch][:, half:half + 1, f, :]

            fgroups = [list(range(g * 16, min(g * 16 + 16, NF)))
                       for g in range((NF + 15) // 16)]
            for fg in fgroups:
                f0, nf = fg[0], len(fg)
                pg = psP.tile([16, 512], F32, tag="psp")
                for fi, f in enumerate(fg):
                    fo = fi * 32
                    if f == 0 or f == 64:
                        ky = 63
                        kh = 0 if f == 0 else 1
                        for ch in range(NCH):
                            nc.tensor.matmul(
                                pg[:, fo:fo + 16],
                                q_slice(ch, f, 0),
                                KSv[ch][:, kh:kh + 1, ky, :],
                                start=(ch == 0), stop=(ch == NCH - 1))
                    else:
                        # 8 accumulating mms: [P_re | P_im] lands directly
                        for ch in range(NCH):
                            nc.tensor.matmul(
                                pg[:, fo:fo + 32],
                                q_slice(ch, f, 0),
                                KSv[ch][:, 1:3, f - 1, :],   # [Kre | -Kim]
                                start=(ch == 0), stop=False)
                        for ch in range(NCH):
                            nc.tensor.matmul(
                                pg[:, fo:fo + 32],
                                q_slice(ch, f, 1),
                                KSv[ch][:, 0:2, f - 1, :],   # [Kim | Kre]
                                start=False, stop=(ch == NCH - 1))
                # drain psum -> PP
                if f0 == 0:
                    nc.any.tensor_copy(PP[:, 0:16], pg[:, 0:16])
                    nc.vector.memset(PP[:, 16:32], 0.0)
                    nc.any.tensor_copy(PP[:, 32:nf * 32], pg[:, 32:nf * 32])
                elif nf == 1:  # f = 64
                    nc.any.tensor_copy(PP[:, f0 * 32:f0 * 32 + 16], pg[:, 0:16])
                    nc.vector.memset(PP[:, f0 * 32 + 16:f0 * 32 + 32], 0.0)
                else:
                    nc.any.tensor_copy(
                        PP[:, f0 * 32:(f0 + nf) * 32], pg[:, 0:nf * 32])
                # skew write + aligned readback, per component
                for comp, SKD in ((0, skd_re), (1, skd_im)):
                    skew_dst = bass.AP(
                        SKD.tensor, 1 + 48 * f0,
                        [[SKROW + 1, 16], [48, nf], [1, 16]])
                    nc.scalar.dma_start(
                        skew_dst, PPv[:, f0:f0 + nf, comp:comp + 1, :])
                    if comp == 0:
                        g0, gn, prow = f0, nf, f0
                    else:
                        g0 = max(f0, 1)
                        gn = min(f0 + nf, 64) - g0
                        prow = 64 + g0
                        if gn <= 0:
                            continue
                    rd_src = bass.AP(
                        SKD.tensor, 48 * g0,
                        [[48, gn], [SKROW, 16], [1, 32]])
                    nc.sync.dma_start(
                        DG4[prow:prow + gn, :].rearrange(
                            "f (a j) -> f a j", j=32),
                        rd_src)

            # ---- fold the 16 a-rows: 4 halving slice-adds in place ----
            w = 256
            while w >= 32:
                nc.vector.tensor_add(DG4[:, 0:w], DG4[:, 0:w], DG4[:, w:2 * w])
                w //= 2
            # DG4[:, 0:16] = LO1, DG4[:, 16:32] = HI (packed-f partitions)
            LO1, HI = DG4[:, 0:16], DG4[:, 16:32]
            # LO2 rows: 0..64 <- im(f), 65..127 <- re(f=1..63); f=0,64 im = 0
            LO2 = smallp.tile([128, 16], F32, tag="lo2")
            nc.vector.memset(LO2[0:1, :], 0.0)
            nc.vector.memset(LO2[64:65, :], 0.0)
            nc.sync.dma_start(LO2[1:64, :], DG4[65:128, 0:16])
            nc.sync.dma_start(LO2[NF:128, :], DG4[1:64, 0:16])

            ps_mv_t = psA.tile([128, 512], F32, tag="psa")
            ps_mv = ps_mv_t[:, 0:R]
            nc.tensor.matmul(ps_mv, widft_sb[:], HI, start=True, stop=False)
            nc.tensor.matmul(ps_mv, widftl1_sb[:], LO1, start=False, stop=False)
            nc.tensor.matmul(ps_mv, widftl2_sb[:], LO2[:], start=False, stop=True)
            mv_sb = smallp.tile([128, R], F32, tag="mv")
            nc.any.tensor_copy(mv_sb[:], ps_mv)

            # ---- all-reduce scores over batch ----
            mv_dram = dramp.tile([L], F32, tag="mvd")
            sc_dram = dramp.tile([L], F32, tag="scd")
            nc.gpsimd.dma_start(mv_dram.rearrange("(p w) -> p w", w=R), mv_sb[:])
            if no_collective:
                nc.gpsimd.dma_start(sc_dram[:], mv_dram[:])
            else:
                nc.gpsimd.collective_compute(
                    "AllReduce",
                    AL.add,
                    replica_groups=[list(range(NCORES))],
                    ins=[mv_dram.opt()],
                    outs=[sc_dram.opt()],
                )

            # ---- top-7 + softmax weights ----
            sc_sb = smallp.tile([1, L], F32, tag="scsb")
            mvl_sb = smallp.tile([1, L], F32, tag="mvl")
            nc.gpsimd.dma_start(sc_sb[:], sc_dram.rearrange("(o l) -> o l", o=1))
            nc.gpsimd.dma_start(mvl_sb[:], mv_dram.rearrange("(o l) -> o l", o=1))
            mx8 = smallp.tile([1, 8], F32, tag="mx8")
            idx8 = smallp.tile([1, 8], U32, tag="idx8")
            nc.vector.max(mx8[:], sc_sb[:])
            nc.vector.max_index(idx8[:], mx8[:], sc_sb[:])

            _, deltas = nc.values_load_multi_w_load_instructions(
                idx8[0:1, 0:TOPK], min_val=0, max_val=L - 1,
                skip_runtime_bounds_check=True,
                engines=(mybir.EngineType.PE, mybir.EngineType.DVE))

            wv = smallp.tile([1, 8], F32, tag="wv")
            nc.vector.memset(wv[:], 0.0)
            for i in range(TOPK):
                nc.vector.tensor_copy(
                    wv[0:1, i:i + 1], mvl_sb[0:1, bass.ds(deltas[i], 1)])
            nc.scalar.activation(
                wv[0:1, 0:TOPK], wv[0:1, 0:TOPK], mybir.ActivationFunctionType.Exp)
            wsum = smallp.tile([1, 1], F32, tag="wsum")
            nc.vector.reduce_sum(wsum[:], wv[0:1, 0:TOPK], axis=mybir.AxisListType.X)
            wrec = smallp.tile([1, 1], F32, tag="wrec")
            nc.vector.reciprocal(wrec[:], wsum[:])
            nc.vector.tensor_scalar(
                wv[0:1, 0:TOPK], wv[0:1, 0:TOPK], wrec[:], None, AL.mult)
            wb = smallp.tile([128, 8], F32, tag="wb")
            nc.gpsimd.partition_broadcast(wb[:, 0:8], wv[0:1, 0:8])

            # ---- 7-tap weighted shifted sum, PSUM -> DRAM direct ----
            offs = [L - d for d in deltas]
            WIall = constp.tile([128, TOPK * 128], BF16, tag="wiall")
            for i in range(TOPK):
                nc.vector.tensor_scalar(
                    WIall[:, i * 128:(i + 1) * 128], identb_sb[:],
                    wb[:, i:i + 1], None, AL.mult)
            WI = [WIall[:, i * 128:(i + 1) * 128] for i in range(TOPK)]
            # HAM warm-up: PE has idled through the collective; burn ~4us of
            # throwaway matmuls (dependent on WIall, i.e. post-collective) so
            # the real taps run at full clock.
            pw = psP.tile([16, 512], F32, tag="psp")
            for i in range(24):
                nc.tensor.matmul(
                    pw[:, 0:128], WIall[0:128, (i % 7) * 128:(i % 7) * 128 + 16],
                    VT[0][:, (i % 8) * 128:(i % 8) * 128 + 128],
                    start=True, stop=True)
            odst = out_ext.ap().rearrange("(g p) (s w) -> g s p w", p=128, w=512)
            ot_i = 0
            for ks in range(4):
                for ch in range(NCH):
                    pt = psO.tile([128, 512], F32, tag="pso")
                    for i in range(TOPK):
                        nc.tensor.matmul(
                            pt[:], WI[i][:],
                            VT[ch][:, bass.ds(offs[i] + ks * 512, 512)],
                            start=(i == 0), stop=(i == TOPK - 1))
                    ot = stagep.tile([128, 512], F32, tag="ot", bufs=3)
                    _copy_alt(ot_i, ot[:], pt[:])
                    ot_i += 1
                    nc.sync.dma_start(odst[ch, ks], ot[:])

    return nc


_NC_CACHE = {}


def _get_nc():
    if "nc" not in _NC_CACHE:
        nc = bacc.Bacc(
            "TRN2", target_bir_lowering=False, debug=False, num_devices=NCORES)
        build_kernel(nc)
        nc.compile()
        _NC_CACHE["nc"] = nc
    return _NC_CACHE["nc"]


def _in_maps(queries, keys, values):
    wdftq, wdftk, widft, wl1, wl2, identb, skz = _consts()
    maps = []
    for b in range(B):
        q2 = np.ascontiguousarray(queries[b], dtype=np.float32).reshape(L, C)
        k2 = np.ascontiguousarray(keys[b], dtype=np.float32).reshape(L, C)
        v2 = np.ascontiguousarray(values[b], dtype=np.float32).reshape(L, C)
        # [u, r, g, c] -> [g, u, r, c]
        qx = q2.reshape(U, R, NCH, 128).transpose(2, 0, 1, 3)
        kx = k2.reshape(U, R, NCH, 128)[:, ::-1].transpose(2, 0, 1, 3)
        maps.append({
            "qx": np.ascontiguousarray(qx, dtype=np.float16).reshape(NCH * 128, R * 128),
            "kx": np.ascontiguousarray(kx, dtype=np.float16).reshape(NCH * 128, R * 128),
            "vt": np.ascontiguousarray(v2.T, dtype=ml_dtypes.bfloat16),
            "wdftq": wdftq, "wdftk": wdftk,
            "widft": widft, "widftl1": wl1, "widftl2": wl2,
            "identb": identb, "skz": skz,
        })
    return maps


def run(queries, keys, values, trace=False):
    nc = _get_nc()
    res = bass_utils.run_bass_kernel_spmd(
        nc, _in_maps(queries, keys, values),
        core_ids=list(range(NCORES)), trace=trace)
    outs = [res.results[b]["out"].reshape(C, L).T.reshape(L, H, E)
            for b in range(B)]
    return np.stack(outs, axis=0), res


def kernel(queries, keys, values, attn_mask=None):
    out, _ = run(np.asarray(queries), np.asarray(keys), np.asarray(values))
    return out.astype(np.float32)


# revision 13
# speedup vs baseline: 1.1357x; 1.1357x over previous
"""AutoCorrelation (Autoformer) Trainium2 kernel, 8-core data-parallel over batch.

Per core (one batch b), computes mean_value[b, tau] = (1/(H*E)) sum_c
circ-crosscorr(k[:,c], q[:,c])[tau] via a 16-subsequence DFT-128 decomposition
(t = 16u + r), then AllReduce over cores -> top-7 delays -> softmax weights ->
7-tap shifted weighted sum of V.

v2 layout/engine plan (all heavy matmuls fp16/bf16, 1 cycle/row on PE):
  - host prep: q/k pre-gathered per 128-channel chunk as [u, r, c] fp16 (K with
    r reversed, which the diagonal-skew sums require), V pre-transposed [c, t]
    bf16, output left as [c, t] fp32 and un-transposed on host.
  - stage A (spectra): per (ch, r) the data block [u=128, c=128] is the
    stationary operand (128 contiguous fp16 cols -> FWL), streaming a packed
    DFT matrix: Q side 128 cols (re|im), K side 192 cols (im|re|-im). PSUM ->
    fp16 spectra in SBUF laid out [c, (fp, r)] so stage-P slices are contiguous.
  - stage P (cross spectra): per (f, ch) TWO accumulating matmuls produce
    P_re and P_im directly in PSUM:
      [Qre]^T [Kre | -Kim] + [Qim]^T [Kim | Kre] = [P_re | P_im]
    (the -Kim slab comes for free from stage A's K DFT matrix).
  - diagonal sums: DRAM skew write / aligned readback (as before), but the
    16-partition fold is a DVE segmented tensor_reduce over the readback laid
    out [j, (f, a)] - no gpsimd involvement.
  - IDFT-128 (fp32, exact) -> mean_value; AllReduce; on-device top-7 +
    softmax; 7-tap PSUM-accumulated taps on PE with scaled-identity
    stationaries over doubled V^T; output DMA'd straight from PSUM to DRAM.
"""

import os
import sys
import numpy as np

for p in ("/opt/trn_rl_repo",):
    if p not in sys.path and os.path.isdir(p):
        sys.path.insert(0, p)

import ml_dtypes
import concourse.bass as bass
import concourse.bacc as bacc
import concourse.tile as tile
import concourse.mybir as mybir
from concourse import bass_utils

F32 = mybir.dt.float32
F16 = mybir.dt.float16
BF16 = mybir.dt.bfloat16
U32 = mybir.dt.uint32
AL = mybir.AluOpType

B, L, H, E = 8, 2048, 8, 64
C = H * E            # 512
U, R = 128, 16       # L = U * R ; t = 16*u + r
NF = 65              # rfft freqs of DFT-128 (0..64)
NCH = C // 128       # 4 channel chunks
NCORES = 8
TOPK = 7
SKROW = 3121         # skew row pitch (DRAM floats per a-row)


def _consts():
    u = np.arange(U)
    f65 = np.arange(NF)
    f2 = np.arange(1, 64)

    # Q-side packed DFT: cols 0..64 = cos, 65..127 = -sin (f=1..63)
    wdftq = np.zeros((U, 128), dtype=np.float16)
    wdftq[:, :NF] = np.cos(2 * np.pi * np.outer(u, f65) / U)
    wdftq[:, NF:] = -np.sin(2 * np.pi * np.outer(u, f2) / U)

    # K-side packed DFT: cols 0..62 = -sin (im, f=1..63), 63..127 = cos
    # (re, f=0..64), 128..190 = +sin (-im, f=1..63), col 191 zero pad.
    wdftk = np.zeros((U, 192), dtype=np.float16)
    wdftk[:, 0:63] = -np.sin(2 * np.pi * np.outer(u, f2) / U)
    wdftk[:, 63:128] = np.cos(2 * np.pi * np.outer(u, f65) / U)
    wdftk[:, 128:191] = np.sin(2 * np.pi * np.outer(u, f2) / U)

    # IDFT on packed spectrum -> mean_value (incl 2x Hermitian weight, 1/(U*C))
    widft = np.zeros((128, U), dtype=np.float32)
    v = np.arange(U)
    scale = np.ones(NF)
    scale[1:64] = 2.0
    norm = 1.0 / (U * C)
    widft[:NF, :] = (scale[:, None] * np.cos(2 * np.pi * np.outer(f65, v) / U)) * norm
    widft[NF:, :] = (-2.0 * np.sin(2 * np.pi * np.outer(f2, v) / U)) * norm

    twv1 = np.zeros((128, 1), dtype=np.float32)
    twv2 = np.zeros((128, 1), dtype=np.float32)
    twv1[:NF, 0] = np.cos(2 * np.pi * f65 / U)
    twv2[:NF, 0] = -np.sin(2 * np.pi * f65 / U)
    twv1[NF:, 0] = np.cos(2 * np.pi * f2 / U)
    twv2[NF:, 0] = np.sin(2 * np.pi * f2 / U)
    widft_l1 = (twv1 * widft).astype(np.float32)
    widft_l2 = (twv2 * widft).astype(np.float32)

    identb = np.eye(128, dtype=ml_dtypes.bfloat16)
    skz = np.zeros((16 * SKROW,), dtype=np.float32)
    return wdftq, wdftk, widft, widft_l1, widft_l2, identb, skz


def build_kernel(nc, no_collective=False):
    qx_ext = nc.dram_tensor("qx", [NCH * 128, R * 128], F16, kind="ExternalInput")
    kx_ext = nc.dram_tensor("kx", [NCH * 128, R * 128], F16, kind="ExternalInput")
    vt_ext = nc.dram_tensor("vt", [C, L], BF16, kind="ExternalInput")
    wdftq_ext = nc.dram_tensor("wdftq", [U, 128], F16, kind="ExternalInput")
    wdftk_ext = nc.dram_tensor("wdftk", [U, 192], F16, kind="ExternalInput")
    widft_ext = nc.dram_tensor("widft", [128, U], F32, kind="ExternalInput")
    widftl1_ext = nc.dram_tensor("widftl1", [128, U], F32, kind="ExternalInput")
    widftl2_ext = nc.dram_tensor("widftl2", [128, U], F32, kind="ExternalInput")
    identb_ext = nc.dram_tensor("identb", [128, 128], BF16, kind="ExternalInput")
    skz_ext = nc.dram_tensor("skz", [16 * SKROW], F32, kind="ExternalInput")
    out_ext = nc.dram_tensor("out", [C, L], BF16, kind="ExternalOutput")

    with tile.TileContext(nc) as tc:
        with (
            tc.tile_pool(name="const", bufs=1) as constp,
            tc.tile_pool(name="spec", bufs=1) as specp,
            tc.tile_pool(name="stage", bufs=2) as stagep,
            tc.tile_pool(name="vt", bufs=1) as vtp,
            tc.tile_pool(name="small", bufs=1) as smallp,
            tc.tile_pool(name="dg", bufs=2) as dgp,
            tc.tile_pool(name="psA", bufs=4, space="PSUM") as psA,
            tc.tile_pool(name="psP", bufs=2, space="PSUM") as psP,
            tc.tile_pool(name="psO", bufs=2, space="PSUM") as psO,
            tc.tile_pool(name="dram", bufs=1, space="DRAM") as dramp,
        ):
            # ---- constants ----
            wdftq_sb = constp.tile([U, 128], F16, tag="wdftq")
            wdftk_sb = constp.tile([U, 192], F16, tag="wdftk")
            widft_sb = constp.tile([128, U], F32, tag="widft")
            widftl1_sb = constp.tile([128, U], F32, tag="widftl1")
            widftl2_sb = constp.tile([128, U], F32, tag="widftl2")
            identb_sb = constp.tile([128, 128], BF16, tag="identb")
            nc.sync.dma_start(wdftq_sb[:], wdftq_ext.ap())
            nc.sync.dma_start(wdftk_sb[:], wdftk_ext.ap())

            # ---- stage A: subsequence spectra, fp16, layout [c, (fp, r)] ----
            QS = [specp.tile([128, 128 * R], F16, tag=f"qs{ch}", name=f"qs{ch}")
                  for ch in range(NCH)]
            KS = [specp.tile([128, 192 * R], F16, tag=f"ks{ch}", name=f"ks{ch}")
                  for ch in range(NCH)]
            qsrc = qx_ext.ap().rearrange("(g p) w -> g p w", p=128)
            ksrc = kx_ext.ap().rearrange("(g p) w -> g p w", p=128)
            def _copy_alt(i, dst, src):
                if i % 2 == 0:
                    nc.vector.tensor_copy(dst, src)
                else:
                    nc.scalar.activation(
                        dst, src, mybir.ActivationFunctionType.Copy)
            cp_i = 0
            for ch in range(NCH):
                xq = stagep.tile([128, R * 128], F16, tag="xq", name="xq")
                nc.sync.dma_start(xq[:], qsrc[ch])
                for r4 in range(R // 4):
                    ps = psA.tile([128, 512], F32, tag="psa")
                    for j in range(4):
                        r = r4 * 4 + j
                        nc.tensor.matmul(
                            ps[:, j * 128:(j + 1) * 128],
                            xq[:, r * 128:(r + 1) * 128],
                            wdftq_sb[:], start=True, stop=True)
                    _copy_alt(cp_i, QS[ch][:, r4 * 512:(r4 + 1) * 512], ps[:])
                    cp_i += 1
                xk = stagep.tile([128, R * 128], F16, tag="xk", name="xk")
                nc.sync.dma_start(xk[:], ksrc[ch])
                for r2 in range(R // 2):
                    psk_t = psA.tile([128, 512], F32, tag="psa")
                    psk = psk_t[:, 0:384]
                    for j in range(2):
                        r = r2 * 2 + j
                        nc.tensor.matmul(
                            psk[:, j * 192:(j + 1) * 192],
                            xk[:, r * 128:(r + 1) * 128],
                            wdftk_sb[:], start=True, stop=True)
                    _copy_alt(cp_i, KS[ch][:, r2 * 384:(r2 + 1) * 384], psk[:])
                    cp_i += 1

            # views [c, h, y, r]: Q h in {re, im}, K h in {im, re, -im}
            QSv = [QS[ch].rearrange("c (r h y) -> c h y r", h=2, y=64)
                   for ch in range(NCH)]
            KSv = [KS[ch].rearrange("c (r h y) -> c h y r", h=3, y=64)
                   for ch in range(NCH)]

            # skew scratch in DRAM, pre-zeroed from a host zeros buffer
            skd_re = dramp.tile([16 * SKROW], F32, tag="skdre", name="skd_re")
            skd_im = dramp.tile([16 * SKROW], F32, tag="skdim", name="skd_im")
            nc.sync.dma_start(skd_re[:], skz_ext.ap())
            nc.sync.dma_start(skd_im[:], skz_ext.ap())

            # ---- deferred consts + V^T load (scalar queue) ----
            nc.scalar.dma_start(widft_sb[:], widft_ext.ap())
            nc.scalar.dma_start(widftl1_sb[:], widftl1_ext.ap())
            nc.scalar.dma_start(widftl2_sb[:], widftl2_ext.ap())
            nc.scalar.dma_start(identb_sb[:], identb_ext.ap())
            VT = [vtp.tile([128, 2 * L], BF16, tag=f"vt{ch}", name=f"vt{ch}")
                  for ch in range(NCH)]
            vsrc = vt_ext.ap().rearrange("(g p) t -> g p t", p=128)
            for ch in range(NCH):
                nc.scalar.dma_start(VT[ch][:, 0:L], vsrc[ch])
                nc.vector.tensor_copy(VT[ch][:, L:2 * L], VT[ch][:, 0:L])

            # ---- stage P + diagonal sums ----
            # PP[a, f*32 + (0:16 re | 16:32 im)] with a = Q's r, b = K's slot
            PP = specp.tile([16, NF * 32], F32, tag="pp", name="pp")
            PPv = PP.rearrange("a (f h b) -> a f h b", h=2, b=16)
            # readback target: partition = packed f (re 0..64 | im f=1..63 at
            # 64+f), free = (a, j); folding a is then 4 free-dim slice adds.
            DG4 = dgp.tile([128, 16 * 32], F32, tag="dg4", name="dg4", bufs=1)

            def q_slice(ch, f, half):
                # half 0 = re, 1 = im (packed -sin sums)
                if f == 64:
                    assert half == 0
                    return QSv[ch][:, 1:2, 0, :]
                return QSv[ch][:, half:half + 1, f, :]

            fgroups = [list(range(g * 16, min(g * 16 + 16, NF)))
                       for g in range((NF + 15) // 16)]
            for fg in fgroups:
                f0, nf = fg[0], len(fg)
                pg = psP.tile([16, 512], F32, tag="psp")
                for fi, f in enumerate(fg):
                    fo = fi * 32
                    if f == 0 or f == 64:
                        ky = 63
                        kh = 0 if f == 0 else 1
                        for ch in range(NCH):
                            nc.tensor.matmul(
                                pg[:, fo:fo + 16],
                                q_slice(ch, f, 0),
                                KSv[ch][:, kh:kh + 1, ky, :],
                                start=(ch == 0), stop=(ch == NCH - 1))
                    else:
                        # 8 accumulating mms: [P_re | P_im] lands directly
                        for ch in range(NCH):
                            nc.tensor.matmul(
                                pg[:, fo:fo + 32],
                                q_slice(ch, f, 0),
                                KSv[ch][:, 1:3, f - 1, :],   # [Kre | -Kim]
                                start=(ch == 0), stop=False)
                        for ch in range(NCH):
                            nc.tensor.matmul(
                                pg[:, fo:fo + 32],
                                q_slice(ch, f, 1),
                                KSv[ch][:, 0:2, f - 1, :],   # [Kim | Kre]
                                start=False, stop=(ch == NCH - 1))
                # drain psum -> PP
                if f0 == 0:
                    nc.any.tensor_copy(PP[:, 0:16], pg[:, 0:16])
                    nc.vector.memset(PP[:, 16:32], 0.0)
                    nc.any.tensor_copy(PP[:, 32:nf * 32], pg[:, 32:nf * 32])
                elif nf == 1:  # f = 64
                    nc.any.tensor_copy(PP[:, f0 * 32:f0 * 32 + 16], pg[:, 0:16])
                    nc.vector.memset(PP[:, f0 * 32 + 16:f0 * 32 + 32], 0.0)
                else:
                    nc.any.tensor_copy(
                        PP[:, f0 * 32:(f0 + nf) * 32], pg[:, 0:nf * 32])
                # skew write + aligned readback, per component
                for comp, SKD in ((0, skd_re), (1, skd_im)):
                    skew_dst = bass.AP(
                        SKD.tensor, 1 + 48 * f0,
                        [[SKROW + 1, 16], [48, nf], [1, 16]])
                    nc.scalar.dma_start(
                        skew_dst, PPv[:, f0:f0 + nf, comp:comp + 1, :])
                    if comp == 0:
                        g0, gn, prow = f0, nf, f0
                    else:
                        g0 = max(f0, 1)
                        gn = min(f0 + nf, 64) - g0
                        prow = 64 + g0
                        if gn <= 0:
                            continue
                    rd_src = bass.AP(
                        SKD.tensor, 48 * g0,
                        [[48, gn], [SKROW, 16], [1, 32]])
                    rd_eng = nc.sync if comp == 0 else nc.gpsimd
                    rd_eng.dma_start(
                        DG4[prow:prow + gn, :].rearrange(
                            "f (a j) -> f a j", j=32),
                        rd_src)

            # ---- fold the 16 a-rows: 4 halving slice-adds in place ----
            w = 256
            while w >= 32:
                nc.vector.tensor_add(DG4[:, 0:w], DG4[:, 0:w], DG4[:, w:2 * w])
                w //= 2
            # DG4[:, 0:16] = LO1, DG4[:, 16:32] = HI (packed-f partitions)
            LO1, HI = DG4[:, 0:16], DG4[:, 16:32]
            # LO2 rows: 0..64 <- im(f), 65..127 <- re(f=1..63); f=0,64 im = 0
            LO2 = smallp.tile([128, 16], F32, tag="lo2")
            nc.vector.memset(LO2[0:1, :], 0.0)
            nc.vector.memset(LO2[64:65, :], 0.0)
            nc.sync.dma_start(LO2[1:64, :], DG4[65:128, 0:16])
            nc.sync.dma_start(LO2[NF:128, :], DG4[1:64, 0:16])

            ps_mv_t = psA.tile([128, 512], F32, tag="psa")
            ps_mv = ps_mv_t[:, 0:R]
            nc.tensor.matmul(ps_mv, widft_sb[:], HI, start=True, stop=False)
            nc.tensor.matmul(ps_mv, widftl1_sb[:], LO1, start=False, stop=False)
            nc.tensor.matmul(ps_mv, widftl2_sb[:], LO2[:], start=False, stop=True)
            mv_sb = smallp.tile([128, R], F32, tag="mv")
            nc.any.tensor_copy(mv_sb[:], ps_mv)

            # ---- all-reduce scores over batch ----
            mv_dram = dramp.tile([L], F32, tag="mvd")
            sc_dram = dramp.tile([L], F32, tag="scd")
            nc.gpsimd.dma_start(mv_dram.rearrange("(p w) -> p w", w=R), mv_sb[:])
            if no_collective:
                nc.gpsimd.dma_start(sc_dram[:], mv_dram[:])
            else:
                nc.gpsimd.collective_compute(
                    "AllReduce",
                    AL.add,
                    replica_groups=[list(range(NCORES))],
                    ins=[mv_dram.opt()],
                    outs=[sc_dram.opt()],
                )

            # ---- top-7 + softmax weights ----
            sc_sb = smallp.tile([1, L], F32, tag="scsb")
            mvl_sb = smallp.tile([1, L], F32, tag="mvl")
            nc.gpsimd.dma_start(sc_sb[:], sc_dram.rearrange("(o l) -> o l", o=1))
            nc.gpsimd.dma_start(mvl_sb[:], mv_dram.rearrange("(o l) -> o l", o=1))
            mx8 = smallp.tile([1, 8], F32, tag="mx8")
            idx8 = smallp.tile([1, 8], U32, tag="idx8")
            nc.vector.max(mx8[:], sc_sb[:])
            nc.vector.max_index(idx8[:], mx8[:], sc_sb[:])

            _, deltas = nc.values_load_multi_w_load_instructions(
                idx8[0:1, 0:TOPK], min_val=0, max_val=L - 1,
                skip_runtime_bounds_check=True,
                engines=(mybir.EngineType.PE, mybir.EngineType.DVE))

            wv = smallp.tile([1, 8], F32, tag="wv")
            nc.vector.memset(wv[:], 0.0)
            for i in range(TOPK):
                nc.vector.tensor_copy(
                    wv[0:1, i:i + 1], mvl_sb[0:1, bass.ds(deltas[i], 1)])
            nc.scalar.activation(
                wv[0:1, 0:TOPK], wv[0:1, 0:TOPK], mybir.ActivationFunctionType.Exp)
            wsum = smallp.tile([1, 1], F32, tag="wsum")
            nc.vector.reduce_sum(wsum[:], wv[0:1, 0:TOPK], axis=mybir.AxisListType.X)
            wrec = smallp.tile([1, 1], F32, tag="wrec")
            nc.vector.reciprocal(wrec[:], wsum[:])
            nc.vector.tensor_scalar(
                wv[0:1, 0:TOPK], wv[0:1, 0:TOPK], wrec[:], None, AL.mult)
            wb = smallp.tile([128, 8], F32, tag="wb")
            nc.gpsimd.partition_broadcast(wb[:, 0:8], wv[0:1, 0:8])

            # ---- 7-tap weighted shifted sum, PSUM -> DRAM direct ----
            offs = [L - d for d in deltas]
            WIall = constp.tile([128, TOPK * 128], BF16, tag="wiall")
            for i in range(TOPK):
                nc.vector.tensor_scalar(
                    WIall[:, i * 128:(i + 1) * 128], identb_sb[:],
                    wb[:, i:i + 1], None, AL.mult)
            WI = [WIall[:, i * 128:(i + 1) * 128] for i in range(TOPK)]
            odst = out_ext.ap().rearrange("(g p) (s w) -> g s p w", p=128, w=512)
            for ks in range(4):
                for ch in (0, 1):
                    pt = psO.tile([128, 512], F32, tag="pso")
                    for i in range(TOPK):
                        nc.tensor.matmul(
                            pt[:], WI[i][:],
                            VT[ch][:, bass.ds(offs[i] + ks * 512, 512)],
                            start=(i == 0), stop=(i == TOPK - 1))
                    ot = stagep.tile([128, 512], BF16, tag="ot", bufs=3)
                    nc.scalar.activation(
                        ot[:], pt[:], mybir.ActivationFunctionType.Copy)
                    nc.sync.dma_start(odst[ch, ks], ot[:])
            # DVE half: bf16 FMA chains (tensor_scalar 4x, then 2x FMAs)
            ovdst = out_ext.ap().rearrange("(g p) t -> g p t", p=128)
            for ch in (2, 3):
                acc = stagep.tile([128, L], BF16, tag=f"acc{ch}", bufs=1)
                nc.vector.tensor_scalar(
                    acc[:], VT[ch][:, bass.ds(offs[0], L)],
                    wb[:, 0:1], None, AL.mult)
                for i in range(1, TOPK):
                    nc.vector.scalar_tensor_tensor(
                        acc[:], VT[ch][:, bass.ds(offs[i], L)],
                        wb[:, i:i + 1], acc[:], AL.mult, AL.add)
                nc.sync.dma_start(ovdst[ch], acc[:])

    return nc


_NC_CACHE = {}


def _get_nc():
    if "nc" not in _NC_CACHE:
        nc = bacc.Bacc(
            "TRN2", target_bir_lowering=False, debug=False, num_devices=NCORES)
        build_kernel(nc)
        nc.compile()
        _NC_CACHE["nc"] = nc
    return _NC_CACHE["nc"]


def _in_maps(queries, keys, values):
    wdftq, wdftk, widft, wl1, wl2, identb, skz = _consts()
    maps = []
    for b in range(B):
        q2 = np.ascontiguousarray(queries[b], dtype=np.float32).reshape(L, C)
        k2 = np.ascontiguousarray(keys[b], dtype=np.float32).reshape(L, C)
        v2 = np.ascontiguousarray(values[b], dtype=np.float32).reshape(L, C)
        # [u, r, g, c] -> [g, u, r, c]
        qx = q2.reshape(U, R, NCH, 128).transpose(2, 0, 1, 3)
        kx = k2.reshape(U, R, NCH, 128)[:, ::-1].transpose(2, 0, 1, 3)
        maps.append({
            "qx": np.ascontiguousarray(qx, dtype=np.float16).reshape(NCH * 128, R * 128),
            "kx": np.ascontiguousarray(kx, dtype=np.float16).reshape(NCH * 128, R * 128),
            "vt": np.ascontiguousarray(v2.T, dtype=ml_dtypes.bfloat16),
            "wdftq": wdftq, "wdftk": wdftk,
            "widft": widft, "widftl1": wl1, "widftl2": wl2,
            "identb": identb, "skz": skz,
        })
    return maps


def run(queries, keys, values, trace=False):
    nc = _get_nc()
    res = bass_utils.run_bass_kernel_spmd(
        nc, _in_maps(queries, keys, values),
        core_ids=list(range(NCORES)), trace=trace)
    outs = [np.asarray(res.results[b]["out"], dtype=np.float32)
            .reshape(C, L).T.reshape(L, H, E) for b in range(B)]
    return np.stack(outs, axis=0), res


def kernel(queries, keys, values, attn_mask=None):
    out, _ = run(np.asarray(queries), np.asarray(keys), np.asarray(values))
    return out.astype(np.float32)


# revision 14
# speedup vs baseline: 1.1609x; 1.0222x over previous
"""AutoCorrelation (Autoformer) Trainium2 kernel, 8-core data-parallel over batch.

Per core (one batch b), computes mean_value[b, tau] = (1/(H*E)) sum_c
circ-crosscorr(k[:,c], q[:,c])[tau] via a 16-subsequence DFT-128 decomposition
(t = 16u + r), then AllReduce over cores -> top-7 delays -> softmax weights ->
7-tap shifted weighted sum of V.

v2 layout/engine plan (all heavy matmuls fp16/bf16, 1 cycle/row on PE):
  - host prep: q/k pre-gathered per 128-channel chunk as [u, r, c] fp16 (K with
    r reversed, which the diagonal-skew sums require), V pre-transposed [c, t]
    bf16, output left as [c, t] fp32 and un-transposed on host.
  - stage A (spectra): per (ch, r) the data block [u=128, c=128] is the
    stationary operand (128 contiguous fp16 cols -> FWL), streaming a packed
    DFT matrix: Q side 128 cols (re|im), K side 192 cols (im|re|-im). PSUM ->
    fp16 spectra in SBUF laid out [c, (fp, r)] so stage-P slices are contiguous.
  - stage P (cross spectra): per (f, ch) TWO accumulating matmuls produce
    P_re and P_im directly in PSUM:
      [Qre]^T [Kre | -Kim] + [Qim]^T [Kim | Kre] = [P_re | P_im]
    (the -Kim slab comes for free from stage A's K DFT matrix).
  - diagonal sums: DRAM skew write / aligned readback (as before), but the
    16-partition fold is a DVE segmented tensor_reduce over the readback laid
    out [j, (f, a)] - no gpsimd involvement.
  - IDFT-128 (fp32, exact) -> mean_value; AllReduce; on-device top-7 +
    softmax; 7-tap PSUM-accumulated taps on PE with scaled-identity
    stationaries over doubled V^T; output DMA'd straight from PSUM to DRAM.
"""

import os
import sys
import numpy as np

for p in ("/opt/trn_rl_repo",):
    if p not in sys.path and os.path.isdir(p):
        sys.path.insert(0, p)

import ml_dtypes
import concourse.bass as bass
import concourse.bacc as bacc
import concourse.tile as tile
import concourse.mybir as mybir
from concourse import bass_utils

F32 = mybir.dt.float32
F16 = mybir.dt.float16
BF16 = mybir.dt.bfloat16
U32 = mybir.dt.uint32
AL = mybir.AluOpType

B, L, H, E = 8, 2048, 8, 64
C = H * E            # 512
U, R = 128, 16       # L = U * R ; t = 16*u + r
NF = 65              # rfft freqs of DFT-128 (0..64)
NCH = C // 128       # 4 channel chunks
NCORES = 8
TOPK = 7
SKROW = 3121         # skew row pitch (DRAM floats per a-row)


def _consts():
    u = np.arange(U)
    f65 = np.arange(NF)
    f2 = np.arange(1, 64)

    # Q-side packed DFT: cols 0..64 = cos, 65..127 = -sin (f=1..63)
    wdftq = np.zeros((U, 128), dtype=np.float16)
    wdftq[:, :NF] = np.cos(2 * np.pi * np.outer(u, f65) / U)
    wdftq[:, NF:] = -np.sin(2 * np.pi * np.outer(u, f2) / U)

    # K-side packed DFT: cols 0..62 = -sin (im, f=1..63), 63..127 = cos
    # (re, f=0..64), 128..190 = +sin (-im, f=1..63), col 191 zero pad.
    wdftk = np.zeros((U, 192), dtype=np.float16)
    wdftk[:, 0:63] = -np.sin(2 * np.pi * np.outer(u, f2) / U)
    wdftk[:, 63:128] = np.cos(2 * np.pi * np.outer(u, f65) / U)
    wdftk[:, 128:191] = np.sin(2 * np.pi * np.outer(u, f2) / U)

    # IDFT on packed spectrum -> mean_value (incl 2x Hermitian weight, 1/(U*C))
    widft = np.zeros((128, U), dtype=np.float32)
    v = np.arange(U)
    scale = np.ones(NF)
    scale[1:64] = 2.0
    norm = 1.0 / (U * C)
    widft[:NF, :] = (scale[:, None] * np.cos(2 * np.pi * np.outer(f65, v) / U)) * norm
    widft[NF:, :] = (-2.0 * np.sin(2 * np.pi * np.outer(f2, v) / U)) * norm

    twv1 = np.zeros((128, 1), dtype=np.float32)
    twv2 = np.zeros((128, 1), dtype=np.float32)
    twv1[:NF, 0] = np.cos(2 * np.pi * f65 / U)
    twv2[:NF, 0] = -np.sin(2 * np.pi * f65 / U)
    twv1[NF:, 0] = np.cos(2 * np.pi * f2 / U)
    twv2[NF:, 0] = np.sin(2 * np.pi * f2 / U)
    widft_l1 = (twv1 * widft).astype(np.float32)
    widft_l2 = (twv2 * widft).astype(np.float32)

    identb = np.eye(128, dtype=ml_dtypes.bfloat16)
    skz = np.zeros((16 * SKROW,), dtype=np.float32)
    return wdftq, wdftk, widft, widft_l1, widft_l2, identb, skz


def build_kernel(nc, no_collective=False):
    qx_ext = nc.dram_tensor("qx", [NCH * 128, R * 128], F16, kind="ExternalInput")
    kx_ext = nc.dram_tensor("kx", [NCH * 128, R * 128], F16, kind="ExternalInput")
    vt_ext = nc.dram_tensor("vt", [C, L], BF16, kind="ExternalInput")
    wdftq_ext = nc.dram_tensor("wdftq", [U, 128], F16, kind="ExternalInput")
    wdftk_ext = nc.dram_tensor("wdftk", [U, 192], F16, kind="ExternalInput")
    widft_ext = nc.dram_tensor("widft", [128, U], F32, kind="ExternalInput")
    widftl1_ext = nc.dram_tensor("widftl1", [128, U], F32, kind="ExternalInput")
    widftl2_ext = nc.dram_tensor("widftl2", [128, U], F32, kind="ExternalInput")
    identb_ext = nc.dram_tensor("identb", [128, 128], BF16, kind="ExternalInput")
    skz_ext = nc.dram_tensor("skz", [16 * SKROW], F32, kind="ExternalInput")
    out_ext = nc.dram_tensor("out", [C, L], BF16, kind="ExternalOutput")

    with tile.TileContext(nc) as tc:
        with (
            tc.tile_pool(name="const", bufs=1) as constp,
            tc.tile_pool(name="spec", bufs=1) as specp,
            tc.tile_pool(name="stage", bufs=2) as stagep,
            tc.tile_pool(name="vt", bufs=1) as vtp,
            tc.tile_pool(name="small", bufs=1) as smallp,
            tc.tile_pool(name="dg", bufs=2) as dgp,
            tc.tile_pool(name="psA", bufs=4, space="PSUM") as psA,
            tc.tile_pool(name="psP", bufs=2, space="PSUM") as psP,
            tc.tile_pool(name="psO", bufs=2, space="PSUM") as psO,
            tc.tile_pool(name="dram", bufs=1, space="DRAM") as dramp,
        ):
            # ---- constants ----
            wdftq_sb = constp.tile([U, 128], F16, tag="wdftq")
            wdftk_sb = constp.tile([U, 192], F16, tag="wdftk")
            widft_sb = constp.tile([128, U], F32, tag="widft")
            widftl1_sb = constp.tile([128, U], F32, tag="widftl1")
            widftl2_sb = constp.tile([128, U], F32, tag="widftl2")
            identb_sb = constp.tile([128, 128], BF16, tag="identb")
            nc.sync.dma_start(wdftq_sb[:], wdftq_ext.ap())
            nc.sync.dma_start(wdftk_sb[:], wdftk_ext.ap())

            # ---- stage A: subsequence spectra, fp16, layout [c, (fp, r)] ----
            QS = [specp.tile([128, 128 * R], F16, tag=f"qs{ch}", name=f"qs{ch}")
                  for ch in range(NCH)]
            KS = [specp.tile([128, 192 * R], F16, tag=f"ks{ch}", name=f"ks{ch}")
                  for ch in range(NCH)]
            qsrc = qx_ext.ap().rearrange("(g p) w -> g p w", p=128)
            ksrc = kx_ext.ap().rearrange("(g p) w -> g p w", p=128)
            def _copy_alt(i, dst, src):
                if i % 2 == 0:
                    nc.vector.tensor_copy(dst, src)
                else:
                    nc.scalar.activation(
                        dst, src, mybir.ActivationFunctionType.Copy)
            cp_i = 0
            for ch in range(NCH):
                xq = stagep.tile([128, R * 128], F16, tag="xq", name="xq")
                nc.sync.dma_start(xq[:], qsrc[ch])
                for r4 in range(R // 4):
                    ps = psA.tile([128, 512], F32, tag="psa")
                    for j in range(4):
                        r = r4 * 4 + j
                        nc.tensor.matmul(
                            ps[:, j * 128:(j + 1) * 128],
                            xq[:, r * 128:(r + 1) * 128],
                            wdftq_sb[:], start=True, stop=True)
                    _copy_alt(cp_i, QS[ch][:, r4 * 512:(r4 + 1) * 512], ps[:])
                    cp_i += 1
                xk = stagep.tile([128, R * 128], F16, tag="xk", name="xk")
                nc.sync.dma_start(xk[:], ksrc[ch])
                for r2 in range(R // 2):
                    psk_t = psA.tile([128, 512], F32, tag="psa")
                    psk = psk_t[:, 0:384]
                    for j in range(2):
                        r = r2 * 2 + j
                        nc.tensor.matmul(
                            psk[:, j * 192:(j + 1) * 192],
                            xk[:, r * 128:(r + 1) * 128],
                            wdftk_sb[:], start=True, stop=True)
                    _copy_alt(cp_i, KS[ch][:, r2 * 384:(r2 + 1) * 384], psk[:])
                    cp_i += 1

            # views [c, h, y, r]: Q h in {re, im}, K h in {im, re, -im}
            QSv = [QS[ch].rearrange("c (r h y) -> c h y r", h=2, y=64)
                   for ch in range(NCH)]
            KSv = [KS[ch].rearrange("c (r h y) -> c h y r", h=3, y=64)
                   for ch in range(NCH)]

            # skew scratch in DRAM, pre-zeroed from a host zeros buffer
            skd_re = dramp.tile([16 * SKROW], F32, tag="skdre", name="skd_re")
            skd_im = dramp.tile([16 * SKROW], F32, tag="skdim", name="skd_im")
            nc.sync.dma_start(skd_re[:], skz_ext.ap())
            nc.sync.dma_start(skd_im[:], skz_ext.ap())

            # ---- deferred consts + V^T load (gpsimd queue) ----
            nc.gpsimd.dma_start(widft_sb[:], widft_ext.ap())
            nc.gpsimd.dma_start(widftl1_sb[:], widftl1_ext.ap())
            nc.gpsimd.dma_start(widftl2_sb[:], widftl2_ext.ap())
            nc.gpsimd.dma_start(identb_sb[:], identb_ext.ap())
            VT = [vtp.tile([128, 2 * L], BF16, tag=f"vt{ch}", name=f"vt{ch}")
                  for ch in range(NCH)]
            vsrc = vt_ext.ap().rearrange("(g p) t -> g p t", p=128)
            for ch in range(NCH):
                nc.gpsimd.dma_start(VT[ch][:, 0:L], vsrc[ch])
                nc.vector.tensor_copy(VT[ch][:, L:2 * L], VT[ch][:, 0:L])

            # ---- stage P + diagonal sums ----
            # PP[a, f*32 + (0:16 re | 16:32 im)] with a = Q's r, b = K's slot
            PP = specp.tile([16, NF * 32], F32, tag="pp", name="pp")
            PPv = PP.rearrange("a (f h b) -> a f h b", h=2, b=16)
            # readback target: partition = packed f (re 0..64 | im f=1..63 at
            # 64+f), free = (a, j); folding a is then 4 free-dim slice adds.
            DG4 = dgp.tile([128, 16 * 32], F32, tag="dg4", name="dg4", bufs=1)

            def q_slice(ch, f, half):
                # half 0 = re, 1 = im (packed -sin sums)
                if f == 64:
                    assert half == 0
                    return QSv[ch][:, 1:2, 0, :]
                return QSv[ch][:, half:half + 1, f, :]

            fgroups = [list(range(g * 16, min(g * 16 + 16, NF)))
                       for g in range((NF + 15) // 16)]
            for fg in fgroups:
                f0, nf = fg[0], len(fg)
                pg = psP.tile([16, 512], F32, tag="psp")
                for fi, f in enumerate(fg):
                    fo = fi * 32
                    if f == 0 or f == 64:
                        ky = 63
                        kh = 0 if f == 0 else 1
                        for ch in range(NCH):
                            nc.tensor.matmul(
                                pg[:, fo:fo + 16],
                                q_slice(ch, f, 0),
                                KSv[ch][:, kh:kh + 1, ky, :],
                                start=(ch == 0), stop=(ch == NCH - 1))
                    else:
                        # 8 accumulating mms: [P_re | P_im] lands directly
                        for ch in range(NCH):
                            nc.tensor.matmul(
                                pg[:, fo:fo + 32],
                                q_slice(ch, f, 0),
                                KSv[ch][:, 1:3, f - 1, :],   # [Kre | -Kim]
                                start=(ch == 0), stop=False)
                        for ch in range(NCH):
                            nc.tensor.matmul(
                                pg[:, fo:fo + 32],
                                q_slice(ch, f, 1),
                                KSv[ch][:, 0:2, f - 1, :],   # [Kim | Kre]
                                start=False, stop=(ch == NCH - 1))
                # drain psum -> PP
                if f0 == 0:
                    nc.any.tensor_copy(PP[:, 0:16], pg[:, 0:16])
                    nc.vector.memset(PP[:, 16:32], 0.0)
                    nc.any.tensor_copy(PP[:, 32:nf * 32], pg[:, 32:nf * 32])
                elif nf == 1:  # f = 64
                    nc.any.tensor_copy(PP[:, f0 * 32:f0 * 32 + 16], pg[:, 0:16])
                    nc.vector.memset(PP[:, f0 * 32 + 16:f0 * 32 + 32], 0.0)
                else:
                    nc.any.tensor_copy(
                        PP[:, f0 * 32:(f0 + nf) * 32], pg[:, 0:nf * 32])
                # skew write + aligned readback, per component
                for comp, SKD in ((0, skd_re), (1, skd_im)):
                    skew_dst = bass.AP(
                        SKD.tensor, 1 + 48 * f0,
                        [[SKROW + 1, 16], [48, nf], [1, 16]])
                    nc.scalar.dma_start(
                        skew_dst, PPv[:, f0:f0 + nf, comp:comp + 1, :])
                    if comp == 0:
                        g0, gn, prow = f0, nf, f0
                    else:
                        g0 = max(f0, 1)
                        gn = min(f0 + nf, 64) - g0
                        prow = 64 + g0
                        if gn <= 0:
                            continue
                    rd_src = bass.AP(
                        SKD.tensor, 48 * g0,
                        [[48, gn], [SKROW, 16], [1, 32]])
                    nc.sync.dma_start(
                        DG4[prow:prow + gn, :].rearrange(
                            "f (a j) -> f a j", j=32),
                        rd_src)

            # ---- fold the 16 a-rows: 4 halving slice-adds in place ----
            w = 256
            while w >= 32:
                nc.vector.tensor_add(DG4[:, 0:w], DG4[:, 0:w], DG4[:, w:2 * w])
                w //= 2
            # DG4[:, 0:16] = LO1, DG4[:, 16:32] = HI (packed-f partitions)
            LO1, HI = DG4[:, 0:16], DG4[:, 16:32]
            # LO2 rows: 0..64 <- im(f), 65..127 <- re(f=1..63); f=0,64 im = 0
            LO2 = smallp.tile([128, 16], F32, tag="lo2")
            nc.vector.memset(LO2[0:1, :], 0.0)
            nc.vector.memset(LO2[64:65, :], 0.0)
            nc.sync.dma_start(LO2[1:64, :], DG4[65:128, 0:16])
            nc.sync.dma_start(LO2[NF:128, :], DG4[1:64, 0:16])

            ps_mv_t = psA.tile([128, 512], F32, tag="psa")
            ps_mv = ps_mv_t[:, 0:R]
            nc.tensor.matmul(ps_mv, widft_sb[:], HI, start=True, stop=False)
            nc.tensor.matmul(ps_mv, widftl1_sb[:], LO1, start=False, stop=False)
            nc.tensor.matmul(ps_mv, widftl2_sb[:], LO2[:], start=False, stop=True)
            mv_sb = smallp.tile([128, R], F32, tag="mv")
            nc.any.tensor_copy(mv_sb[:], ps_mv)

            # ---- all-reduce scores over batch ----
            mv_dram = dramp.tile([L], F32, tag="mvd")
            sc_dram = dramp.tile([L], F32, tag="scd")
            nc.gpsimd.dma_start(mv_dram.rearrange("(p w) -> p w", w=R), mv_sb[:])
            if no_collective:
                nc.gpsimd.dma_start(sc_dram[:], mv_dram[:])
            else:
                nc.gpsimd.collective_compute(
                    "AllReduce",
                    AL.add,
                    replica_groups=[list(range(NCORES))],
                    ins=[mv_dram.opt()],
                    outs=[sc_dram.opt()],
                )

            # ---- top-7 + softmax weights ----
            sc_sb = smallp.tile([1, L], F32, tag="scsb")
            mvl_sb = smallp.tile([1, L], F32, tag="mvl")
            nc.gpsimd.dma_start(sc_sb[:], sc_dram.rearrange("(o l) -> o l", o=1))
            nc.gpsimd.dma_start(mvl_sb[:], mv_dram.rearrange("(o l) -> o l", o=1))
            mx8 = smallp.tile([1, 8], F32, tag="mx8")
            idx8 = smallp.tile([1, 8], U32, tag="idx8")
            nc.vector.max(mx8[:], sc_sb[:])
            nc.vector.max_index(idx8[:], mx8[:], sc_sb[:])

            _, deltas = nc.values_load_multi_w_load_instructions(
                idx8[0:1, 0:TOPK], min_val=0, max_val=L - 1,
                skip_runtime_bounds_check=True,
                engines=(mybir.EngineType.PE, mybir.EngineType.DVE))

            wv = smallp.tile([1, 8], F32, tag="wv")
            nc.vector.memset(wv[:], 0.0)
            for i in range(TOPK):
                nc.vector.tensor_copy(
                    wv[0:1, i:i + 1], mvl_sb[0:1, bass.ds(deltas[i], 1)])
            nc.scalar.activation(
                wv[0:1, 0:TOPK], wv[0:1, 0:TOPK], mybir.ActivationFunctionType.Exp)
            wsum = smallp.tile([1, 1], F32, tag="wsum")
            nc.vector.reduce_sum(wsum[:], wv[0:1, 0:TOPK], axis=mybir.AxisListType.X)
            wrec = smallp.tile([1, 1], F32, tag="wrec")
            nc.vector.reciprocal(wrec[:], wsum[:])
            nc.vector.tensor_scalar(
                wv[0:1, 0:TOPK], wv[0:1, 0:TOPK], wrec[:], None, AL.mult)
            wb = smallp.tile([128, 8], F32, tag="wb")
            nc.gpsimd.partition_broadcast(wb[:, 0:8], wv[0:1, 0:8])

            # ---- 7-tap weighted shifted sum, PSUM -> DRAM direct ----
            offs = [L - d for d in deltas]
            WIall = constp.tile([128, TOPK * 128], BF16, tag="wiall")
            for i in range(TOPK):
                nc.vector.tensor_scalar(
                    WIall[:, i * 128:(i + 1) * 128], identb_sb[:],
                    wb[:, i:i + 1], None, AL.mult)
            WI = [WIall[:, i * 128:(i + 1) * 128] for i in range(TOPK)]
            odst = out_ext.ap().rearrange("(g p) (s w) -> g s p w", p=128, w=512)
            for ks in range(4):
                for ch in (0, 1, 2):
                    pt = psO.tile([128, 512], F32, tag="pso")
                    for i in range(TOPK):
                        nc.tensor.matmul(
                            pt[:], WI[i][:],
                            VT[ch][:, bass.ds(offs[i] + ks * 512, 512)],
                            start=(i == 0), stop=(i == TOPK - 1))
                    ot = stagep.tile([128, 512], BF16, tag="ot", bufs=3)
                    nc.scalar.activation(
                        ot[:], pt[:], mybir.ActivationFunctionType.Copy)
                    nc.sync.dma_start(odst[ch, ks], ot[:])
            # DVE half: bf16 FMA chains (tensor_scalar 4x, then 2x FMAs)
            ovdst = out_ext.ap().rearrange("(g p) t -> g p t", p=128)
            for ch in (3,):
                acc = stagep.tile([128, L], BF16, tag=f"acc{ch}", bufs=1)
                nc.vector.tensor_scalar(
                    acc[:], VT[ch][:, bass.ds(offs[0], L)],
                    wb[:, 0:1], None, AL.mult)
                for i in range(1, TOPK):
                    nc.vector.scalar_tensor_tensor(
                        acc[:], VT[ch][:, bass.ds(offs[i], L)],
                        wb[:, i:i + 1], acc[:], AL.mult, AL.add)
                nc.sync.dma_start(ovdst[ch], acc[:])

    return nc


_NC_CACHE = {}


def _get_nc():
    if "nc" not in _NC_CACHE:
        nc = bacc.Bacc(
            "TRN2", target_bir_lowering=False, debug=False, num_devices=NCORES)
        build_kernel(nc)
        nc.compile()
        _NC_CACHE["nc"] = nc
    return _NC_CACHE["nc"]


def _in_maps(queries, keys, values):
    wdftq, wdftk, widft, wl1, wl2, identb, skz = _consts()
    maps = []
    for b in range(B):
        q2 = np.ascontiguousarray(queries[b], dtype=np.float32).reshape(L, C)
        k2 = np.ascontiguousarray(keys[b], dtype=np.float32).reshape(L, C)
        v2 = np.ascontiguousarray(values[b], dtype=np.float32).reshape(L, C)
        # [u, r, g, c] -> [g, u, r, c]
        qx = q2.reshape(U, R, NCH, 128).transpose(2, 0, 1, 3)
        kx = k2.reshape(U, R, NCH, 128)[:, ::-1].transpose(2, 0, 1, 3)
        maps.append({
            "qx": np.ascontiguousarray(qx, dtype=np.float16).reshape(NCH * 128, R * 128),
            "kx": np.ascontiguousarray(kx, dtype=np.float16).reshape(NCH * 128, R * 128),
            "vt": np.ascontiguousarray(v2.T, dtype=ml_dtypes.bfloat16),
            "wdftq": wdftq, "wdftk": wdftk,
            "widft": widft, "widftl1": wl1, "widftl2": wl2,
            "identb": identb, "skz": skz,
        })
    return maps


def run(queries, keys, values, trace=False):
    nc = _get_nc()
    res = bass_utils.run_bass_kernel_spmd(
        nc, _in_maps(queries, keys, values),
        core_ids=list(range(NCORES)), trace=trace)
    outs = [np.asarray(res.results[b]["out"], dtype=np.float32)
            .reshape(C, L).T.reshape(L, H, E) for b in range(B)]
    return np.stack(outs, axis=0), res


def kernel(queries, keys, values, attn_mask=None):
    out, _ = run(np.asarray(queries), np.asarray(keys), np.asarray(values))
    return out.astype(np.float32)
